# revision 5
# baseline (speedup 1.0000x reference)
"""DCRNN (2x GCNConv + GRU-over-nodes + Linear) on 8 Trainium2 cores.

Strategy (v2)
-------------
* GCN1 uses associativity: h1 = relu((A^T x) W1 + b1).  The A-contraction
  has only IN_FEAT=64 output rows, so pairs of node K-tiles run as two
  concurrent 64-wide column-group matmuls (tile_position col tiling); the
  64->256 projection contracts the stacked [W1;W1] so the two halves sum
  for free.  GCN1 is DMA-bound (~25 MB of A strip).
* h1 columns are computed in two segments (512 / 738+pad).  Each segment's
  XW2 = h1 @ W2 shard is AllGather'd early (fp16), overlapping the rest of
  GCN1 and the local-tile part of GCN2, so the PE never waits long.
* GCN2 contracts 90 K-tiles in the order: own-local tiles (no gather dep),
  gather-1 tiles, gather-2 tiles.  The host builds the A strip with the
  matching permuted row order (gathered layout), own rows zeroed in the
  gathered blocks.
* GRU over the node sequence: 8 Jacobi fixed-point sweeps; gates via
  matmul + pointwise, the h recurrence applied exactly with the DVE
  affine-scan, split into 3 chained chunks so the next sweep's gate
  matmuls pipeline with the scans.  A 64-row halo decouples the cores.
* Final Linear on the node shard; host concatenates the 8 shards.
"""

import numpy as np

NUM_NODES = 10000
IN_FEAT = 64
HID = 256
OUT = 3
CORES = 8
ROWS = NUM_NODES // CORES          # 1250
HALO = 64
L = ROWS + HALO                    # 1314 local sequence length
SWEEPS = 8
KP = 128

S1 = 512                           # first h1 column segment
S2 = ROWS - S1                     # 738 (padded to 768)
S2P = 768
NT1 = (NUM_NODES + 255) // 256     # 40 paired node tiles for GCN1 (10240)
G1T = S1 // KP                     # 4 tiles/core in gather-1
G2T = S2P // KP                    # 6 tiles/core in gather-2
MG1 = CORES * G1T                  # 32 gather-1 K-tiles
MG2 = CORES * G2T                  # 48 gather-2 K-tiles
NLOC = 10                          # local-block K-tiles (1280 rows)
MT2 = NLOC + MG1 + MG2             # 90 K-tiles for GCN2

_CACHE = {}


def _chunks(total, step=512):
    return [(c, min(c + step, total)) for c in range(0, total, step)]


def build_program():
    import concourse.bass as bass
    import concourse.mybir as mybir
    import concourse.tile as tile
    from concourse import bacc

    f16 = mybir.dt.float16
    f32 = mybir.dt.float32
    AF = mybir.ActivationFunctionType
    ALU = mybir.AluOpType

    nc = bacc.Bacc("TRN2", num_devices=CORES)

    # ---- inputs ----
    # a1: GCN1 stream, natural node order, 2-way K-tile interleave:
    # [k2*128+p, j, c] = A_T[node k2*256+j*128+p, own col c]
    a1_d = nc.dram_tensor("a1", [NT1 * KP, 2, ROWS], f16, kind="ExternalInput")
    # a2: GCN2 stream, permuted row order [local | gather1 | gather2],
    # halo columns included.
    a2_d = nc.dram_tensor("a2", [MT2 * KP, L], f16, kind="ExternalInput")
    # x node-major, paired like a1 rows: row (k2*128+p), col (j*64+f)
    xn_d = nc.dram_tensor("xn", [NT1 * KP, 2 * IN_FEAT], f16, kind="ExternalInput")
    w1dup_d = nc.dram_tensor("w1dup", [KP, HID], f16, kind="ExternalInput")
    w2_d = nc.dram_tensor("w2", [HID, HID], f16, kind="ExternalInput")
    wiht_d = nc.dram_tensor("wiht", [HID, 3 * HID], f16, kind="ExternalInput")
    whht_d = nc.dram_tensor("whht", [HID, 3 * HID], f16, kind="ExternalInput")
    fcwt_d = nc.dram_tensor("fcwt", [HID, OUT], f16, kind="ExternalInput")
    ident_d = nc.dram_tensor("ident", [KP, KP], f16, kind="ExternalInput")
    b1c_d = nc.dram_tensor("b1c", [KP, 2], f32, kind="ExternalInput")
    b2c_d = nc.dram_tensor("b2c", [KP, 2], f32, kind="ExternalInput")
    gib_d = nc.dram_tensor("gib", [KP, 6], f32, kind="ExternalInput")
    bhn_d = nc.dram_tensor("bhn", [KP, 2], f32, kind="ExternalInput")
    fcb_d = nc.dram_tensor("fcb", [KP, 1], f32, kind="ExternalInput")
    patch_d = nc.dram_tensor("patch", [KP, 12], f32, kind="ExternalInput")
    out_d = nc.dram_tensor("out_t", [OUT, ROWS], f32, kind="ExternalOutput")

    with tile.TileContext(nc) as tc:
        with (
            tc.tile_pool(name="const", bufs=1) as cpool,
            tc.tile_pool(name="big", bufs=1) as big,
            tc.tile_pool(name="tmp", bufs=4) as tpool,
            tc.tile_pool(name="psxw", bufs=2, space="PSUM") as psxw,
            tc.tile_pool(name="dram", bufs=1, space="DRAM") as dpool,
        ):
            # ---- constants ----
            xn_sb = cpool.tile([KP, NT1, 2, IN_FEAT], f16)
            w1dup_sb = cpool.tile([KP, HID], f16)
            w2_sb = cpool.tile([KP, 2, HID], f16)
            wiht_sb = cpool.tile([KP, 2, 3 * HID], f16)
            whht_sb = cpool.tile([KP, 2, 3 * HID], f16)
            fcwt_sb = cpool.tile([KP, 2, OUT], f16)
            ident_sb = cpool.tile([KP, KP], f16)
            b1c_sb = cpool.tile([KP, 2], f32)
            b2c_sb = cpool.tile([KP, 2], f32)
            gib_sb = cpool.tile([KP, 6], f32)
            bhn_sb = cpool.tile([KP, 2], f32)
            fcb_sb = cpool.tile([KP, 1], f32)
            patch_sb = cpool.tile([KP, 12], f32)

            nc.sync.dma_start(ident_sb[:], ident_d[:])
            nc.sync.dma_start(w1dup_sb[:], w1dup_d[:])
            for k2 in range(NT1):
                eng = nc.sync if k2 % 2 == 0 else nc.scalar
                eng.dma_start(xn_sb[:, k2, :, :], xn_d[k2 * KP:(k2 + 1) * KP, :])
            for k in range(2):
                nc.sync.dma_start(w2_sb[:, k, :], w2_d[k * KP:(k + 1) * KP, :])
                nc.sync.dma_start(wiht_sb[:, k, :], wiht_d[k * KP:(k + 1) * KP, :])
                nc.sync.dma_start(whht_sb[:, k, :], whht_d[k * KP:(k + 1) * KP, :])
                nc.sync.dma_start(fcwt_sb[:, k, :], fcwt_d[k * KP:(k + 1) * KP, :])
            nc.sync.dma_start(b1c_sb[:], b1c_d[:])
            nc.sync.dma_start(b2c_sb[:], b2c_d[:])
            nc.sync.dma_start(gib_sb[:], gib_d[:])
            nc.sync.dma_start(bhn_sb[:], bhn_d[:])
            nc.sync.dma_start(fcb_sb[:], fcb_d[:])
            nc.sync.dma_start(patch_sb[:], patch_d[:])

            # tiny AllGather to absorb the first-collective ncfw setup cost
            ccw_in = dpool.tile([CORES, 64], f16)
            ccw_out = dpool.tile([CORES * CORES, 64], f16, addr_space="Shared")
            nc.sync.dma_start(ccw_in[0:8, :], ident_sb[0:8, 0:64])
            nc.gpsimd.collective_compute(
                "AllGather", mybir.AluOpType.bypass,
                replica_groups=[list(range(CORES))],
                ins=[ccw_in.opt()], outs=[ccw_out.opt()])

            # PE warm-up burst so the HAM clock-gate opens before GCN1
            for i in range(16):
                psd = psxw.tile([KP, 512], f32, tag="xwps", name=f"warm_{i}")
                nc.tensor.matmul(psd[:, :KP], ident_sb[:], ident_sb[:],
                                 start=True, stop=True)

            # ================= GCN1: ax = A1^T x, two column segments ====
            a1p_cm = tc.tile_pool(name="a1stream", bufs=5)
            a1pool = a1p_cm.__enter__()
            psax_cm = tc.tile_pool(name="psax", bufs=4, space="PSUM")
            psax = psax_cm.__enter__()
            h1t_sb = big.tile([KP, 2, S1 + S2P], f16)
            # zero the S2 pad columns so XW2 of pad rows is 0 (not garbage)
            nc.vector.memset(h1t_sb[:, 0, S1 + S2:S1 + S2P], 0.0)
            nc.vector.memset(h1t_sb[:, 1, S1 + S2:S1 + S2P], 0.0)

            def gcn1_pass(c0, c1, tag):
                cw = c1 - c0
                chs = _chunks(cw)
                axps = [psax.tile([KP, 512], f32, tag="ax",
                                  name=f"ax_{tag}_{i}")
                        for i in range(len(chs))]
                for k2 in range(NT1):
                    at = a1pool.tile([KP, 2, cw], f16, tag="a1")
                    eng = nc.sync if k2 % 2 == 0 else nc.scalar
                    eng.dma_start(at[:], a1_d[k2 * KP:(k2 + 1) * KP, :, c0:c1])
                    for ci, (cc0, cc1) in enumerate(chs):
                        nc.tensor.matmul(
                            axps[ci][0:64, :cc1 - cc0], xn_sb[:, k2, 0, :],
                            at[:, 0, cc0:cc1],
                            start=(k2 == 0), stop=(k2 == NT1 - 1),
                            tile_position=(0, 0), skip_group_check=True)
                        nc.tensor.matmul(
                            axps[ci][64:128, :cc1 - cc0], xn_sb[:, k2, 1, :],
                            at[:, 1, cc0:cc1],
                            start=(k2 == 0), stop=(k2 == NT1 - 1),
                            tile_position=(0, 64), skip_group_check=True)
                # copy ax to SBUF (fp16), project with [W1;W1], relu
                for ci, (cc0, cc1) in enumerate(chs):
                    ccw = cc1 - cc0
                    axs = tpool.tile([KP, 512], f16, tag="axs",
                                     name=f"axs_{tag}_{ci}")
                    nc.scalar.activation(axs[:, :ccw], axps[ci][:, :ccw],
                                         AF.Copy)
                    for mm in range(2):
                        psh = psax.tile([KP, 512], f32, tag="ax",
                                        name=f"psh_{tag}_{ci}_{mm}")
                        nc.tensor.matmul(psh[:, :ccw],
                                         w1dup_sb[:, mm * KP:(mm + 1) * KP],
                                         axs[:, :ccw], start=True, stop=True)
                        nc.scalar.activation(
                            h1t_sb[:, mm, c0 + cc0:c0 + cc1],
                            psh[:, :ccw], AF.Relu,
                            bias=b1c_sb[:, mm:mm + 1])

            gcn1_pass(0, S1, "s1")

            # ---- XW2 segment 1 (tiles 0..3), bounce, gather 1 ----
            xw2l_sb = cpool.tile([KP, NLOC, HID], f16)
            bounce1 = dpool.tile([S1, HID], f16)
            bounce2 = dpool.tile([S2P, HID], f16)
            gath1 = dpool.tile([CORES * S1, HID], f16, addr_space="Shared")
            gath2 = dpool.tile([CORES * S2P, HID], f16, addr_space="Shared")

            def xw2_tiles(t0, t1):
                for t in range(t0, t1):
                    ps = psxw.tile([KP, 512], f32, tag="xwps", name=f"xw2_{t}")
                    for k in range(2):
                        nc.tensor.matmul(ps[:, :HID],
                                         h1t_sb[:, k, t * KP:(t + 1) * KP],
                                         w2_sb[:, k, :],
                                         start=(k == 0), stop=(k == 1))
                    if t % 2 == 0:
                        nc.scalar.activation(xw2l_sb[:, t, :], ps[:, :HID],
                                             AF.Copy)
                    else:
                        nc.vector.tensor_copy(xw2l_sb[:, t, :], ps[:, :HID])

            xw2_tiles(0, G1T)
            for t in range(G1T):
                nc.sync.dma_start(bounce1[t * KP:(t + 1) * KP, :],
                                  xw2l_sb[:, t, :])
            nc.gpsimd.collective_compute(
                "AllGather", mybir.AluOpType.bypass,
                replica_groups=[list(range(CORES))],
                ins=[bounce1.opt()], outs=[gath1.opt()])
            # load gathered XW2 back on the gpsimd queue right behind the
            # collective (sync/scalar queues are busy with the A streams)
            xg1_sb = big.tile([KP, MG1, HID], f16)
            xg2_sb = big.tile([KP, MG2, HID], f16)
            for t in range(MG1):
                nc.gpsimd.dma_start(xg1_sb[:, t, :],
                                    gath1[t * KP:(t + 1) * KP, :])

            # ---- GCN1 second column segment, XW2 tiles 4..9, gather 2 ----
            gcn1_pass(S1, ROWS, "s2")
            xw2_tiles(G1T, NLOC)
            for t in range(G2T):
                nc.sync.dma_start(bounce2[t * KP:(t + 1) * KP, :],
                                  xw2l_sb[:, G1T + t, :])
            nc.gpsimd.collective_compute(
                "AllGather", mybir.AluOpType.bypass,
                replica_groups=[list(range(CORES))],
                ins=[bounce2.opt()], outs=[gath2.opt()])

            for t in range(MG2):
                nc.gpsimd.dma_start(xg2_sb[:, t, :],
                                    gath2[t * KP:(t + 1) * KP, :])

            a1p_cm.__exit__(None, None, None)
            psax_cm.__exit__(None, None, None)

            # ================= GCN2 over the halo shard ==================
            psG_cm = tc.tile_pool(name="psG", bufs=1, space="PSUM")
            psG = psG_cm.__enter__()
            a2p_cm = tc.tile_pool(name="a2stream", bufs=12)
            a2pool = a2p_cm.__enter__()

            chg2 = _chunks(L)
            ps2 = [[psG.tile([KP, 512], f32, tag=f"G{mm * 3 + ci}",
                             name=f"ps2_{mm}_{ci}")
                    for ci in range(3)] for mm in range(2)]

            def gcn2_k(k, lhs_tile, first, last):
                at = a2pool.tile([KP, L], f16, tag="a2")
                eng = nc.sync if k % 2 == 0 else nc.scalar
                eng.dma_start(at[:], a2_d[k * KP:(k + 1) * KP, :])
                for mm in range(2):
                    lhsT = lhs_tile(mm)
                    for ci, (c0, c1) in enumerate(chg2):
                        nc.tensor.matmul(ps2[mm][ci][:, :c1 - c0], lhsT,
                                         at[:, c0:c1], start=first, stop=last)

            for t in range(NLOC):
                gcn2_k(t, lambda mm, t=t: xw2l_sb[:, t, mm * KP:(mm + 1) * KP],
                       t == 0, False)
            for t in range(MG1):
                gcn2_k(NLOC + t,
                       lambda mm, t=t: xg1_sb[:, t, mm * KP:(mm + 1) * KP],
                       False, False)
            for t in range(MG2):
                gcn2_k(NLOC + MG1 + t,
                       lambda mm, t=t: xg2_sb[:, t, mm * KP:(mm + 1) * KP],
                       False, t == MG2 - 1)

            h2t_sb = big.tile([KP, 2, L], f16)
            for mm in range(2):
                for ci, (c0, c1) in enumerate(chg2):
                    nc.scalar.activation(h2t_sb[:, mm, c0:c1],
                                         ps2[mm][ci][:, :c1 - c0], AF.Relu,
                                         bias=b2c_sb[:, mm:mm + 1])

            psG_cm.__exit__(None, None, None)
            a2p_cm.__exit__(None, None, None)
            psg_cm = tc.tile_pool(name="ps", bufs=1, space="PSUM")
            pspool = psg_cm.__enter__()

            # ---- GI = W_ih @ h2T + (b_ih [+ b_hh for r,z]) ----
            ch512 = _chunks(L)
            gi_sb = big.tile([KP, 6, L], f16)
            for c0, c1 in ch512:
                psg = [pspool.tile([KP, 512], f32, tag=f"g{m}",
                                   name=f"psgi_{m}") for m in range(6)]
                for m in range(6):
                    for k in range(2):
                        nc.tensor.matmul(psg[m][:, :c1 - c0],
                                         wiht_sb[:, k, m * KP:(m + 1) * KP],
                                         h2t_sb[:, k, c0:c1],
                                         start=(k == 0), stop=(k == 1))
                    if m % 2 == 0:
                        nc.scalar.activation(gi_sb[:, m, c0:c1],
                                             psg[m][:, :c1 - c0], AF.Identity,
                                             bias=gib_sb[:, m:m + 1])
                    else:
                        nc.vector.tensor_scalar_add(gi_sb[:, m, c0:c1],
                                                    psg[m][:, :c1 - c0],
                                                    gib_sb[:, m:m + 1])
            # per-core GI patch on the first HALO columns
            for m in range(6):
                nc.vector.tensor_scalar(gi_sb[:, m, :HALO], gi_sb[:, m, :HALO],
                                        patch_sb[:, m:m + 1],
                                        patch_sb[:, 6 + m:7 + m],
                                        ALU.mult, ALU.add)

            # ---- GRU fixed-point sweeps, chunk-chained scans ----
            hsh_sb = big.tile([KP, 2, L + 1], f16)
            for mm in range(2):
                nc.vector.memset(hsh_sb[:, mm, :], 0.0)
            for s in range(SWEEPS):
                z_sb = big.tile([KP, 2, L], f16, tag="Z")
                b_sb = big.tile([KP, 2, L], f16, tag="B")
                for ci, (c0, c1) in enumerate(ch512):
                    cw = c1 - c0
                    psg = [pspool.tile([KP, 512], f32, tag=f"g{m}",
                                       name=f"psu_{s}_{m}") for m in range(6)]
                    # r,z: identity-load GI then accumulate W_hh @ h_prev
                    for m in range(4):
                        nc.tensor.matmul(psg[m][:, :cw], ident_sb[:],
                                         gi_sb[:, m, c0:c1],
                                         start=True, stop=False)
                    for m in range(6):
                        for k in range(2):
                            nc.tensor.matmul(psg[m][:, :cw],
                                             whht_sb[:, k, m * KP:(m + 1) * KP],
                                             hsh_sb[:, k, c0:c1],
                                             start=(m >= 4 and k == 0),
                                             stop=(k == 1))
                    for mm in range(2):
                        r_t = tpool.tile([KP, 512], f16, tag="r")
                        t_t = tpool.tile([KP, 512], f16, tag="t")
                        un_t = tpool.tile([KP, 512], f16, tag="un")
                        n_t = tpool.tile([KP, 512], f16, tag="n")
                        nc.scalar.activation(r_t[:, :cw], psg[mm][:, :cw],
                                             AF.Sigmoid)
                        nc.scalar.activation(z_sb[:, mm, c0:c1],
                                             psg[2 + mm][:, :cw], AF.Sigmoid)
                        nc.vector.scalar_tensor_tensor(
                            t_t[:, :cw], psg[4 + mm][:, :cw],
                            bhn_sb[:, mm:mm + 1], r_t[:, :cw],
                            ALU.add, ALU.mult)
                        nc.vector.tensor_add(un_t[:, :cw], t_t[:, :cw],
                                             gi_sb[:, 4 + mm, c0:c1])
                        nc.scalar.activation(n_t[:, :cw], un_t[:, :cw], AF.Tanh)
                        nc.vector.scalar_tensor_tensor(
                            b_sb[:, mm, c0:c1], z_sb[:, mm, c0:c1], 1.0,
                            n_t[:, :cw], ALU.subtract, ALU.mult)
                    # chained chunk scans (exact affine recurrence)
                    for mm in range(2):
                        nc.vector.tensor_tensor_scan(
                            hsh_sb[:, mm, 1 + c0:1 + c1],
                            z_sb[:, mm, c0:c1], b_sb[:, mm, c0:c1],
                            0.0 if ci == 0 else hsh_sb[:, mm, c0:c0 + 1],
                            ALU.mult, ALU.subtract)
                # keep-warm matmul so the PE never sees a full HAM window idle
                psd = psxw.tile([KP, 512], f32, tag="xwps", name=f"dwa_{s}")
                nc.tensor.matmul(psd[:, :512], ident_sb[:],
                                 gi_sb[:, 0, 0:512], start=True, stop=True)

            psg_cm.__exit__(None, None, None)

            # ---- final Linear on the real rows (skip halo) ----
            out_sb = cpool.tile([4, ROWS], f32)
            for c0, c1 in _chunks(ROWS):
                cw = c1 - c0
                psf = psxw.tile([KP, 512], f32, tag="xwps")
                for k in range(2):
                    nc.tensor.matmul(psf[:OUT, :cw], fcwt_sb[:, k, :],
                                     hsh_sb[:, k, HALO + 1 + c0:HALO + 1 + c1],
                                     start=(k == 0), stop=(k == 1))
                nc.scalar.activation(out_sb[:OUT, c0:c1], psf[:OUT, :cw],
                                     AF.Identity, bias=fcb_sb[:OUT, :])
            nc.sync.dma_start(out_d[:], out_sb[:OUT, :])

    nc.compile()
    return nc


def host_prepare(inputs):
    """Build the per-core input maps from the full problem inputs."""
    x = np.asarray(inputs["x"], np.float32)
    ei = np.asarray(inputs["edge_index"])
    W1 = np.asarray(inputs["W1"], np.float32)
    b1 = np.asarray(inputs["b1"], np.float32)
    W2 = np.asarray(inputs["W2"], np.float32)
    b2 = np.asarray(inputs["b2"], np.float32)
    W_ih = np.asarray(inputs["W_ih"], np.float32)
    W_hh = np.asarray(inputs["W_hh"], np.float32)
    b_ih = np.asarray(inputs["b_ih"], np.float32)
    b_hh = np.asarray(inputs["b_hh"], np.float32)
    fc_w = np.asarray(inputs["fc_w"], np.float32)
    fc_b = np.asarray(inputs["fc_b"], np.float32)

    N = NUM_NODES
    src, dst = ei[0].astype(np.int64), ei[1].astype(np.int64)
    deg = np.bincount(dst, minlength=N).astype(np.float64) + 1.0
    dinv = 1.0 / np.sqrt(deg)
    # A_T[s, d] = normalization weight of edge s->d (plus self loops)
    at = np.zeros((N, N), np.float32)
    np.add.at(at, (src, dst), (dinv[src] * dinv[dst]).astype(np.float32))
    idx = np.arange(N)
    at[idx, idx] += (dinv * dinv).astype(np.float32)
    at16 = at.astype(np.float16)
    del at

    # x node-major, 2-way interleaved pairs of K-tiles
    NPAD1 = NT1 * 256                    # 10240
    xn = np.zeros((NPAD1, IN_FEAT), np.float16)
    xn[:N] = x.astype(np.float16)
    xn = np.ascontiguousarray(
        xn.reshape(NT1, 2, KP, IN_FEAT).transpose(0, 2, 1, 3)
    ).reshape(NT1 * KP, 2 * IN_FEAT)

    common = {
        "xn": xn,
        "w1dup": np.concatenate([W1, W1], axis=0).astype(np.float16),
        "w2": W2.astype(np.float16),
        "wiht": W_ih.T.astype(np.float16),
        "whht": W_hh.T.astype(np.float16),
        "fcwt": fc_w.T.astype(np.float16),
        "ident": np.eye(KP, dtype=np.float16),
        "b1c": b1.reshape(2, KP).T.astype(np.float32).copy(),
        "b2c": b2.reshape(2, KP).T.astype(np.float32).copy(),
        "gib": (b_ih + np.concatenate([b_hh[:2 * HID],
                                       np.zeros(HID, np.float32)])
                ).reshape(6, KP).T.astype(np.float32).copy(),
        "bhn": b_hh[2 * HID:].reshape(2, KP).T.astype(np.float32).copy(),
        "fcb": np.concatenate([fc_b, np.zeros(KP - OUT, np.float32)]
                              ).reshape(KP, 1),
    }

    in_maps = []
    for c in range(CORES):
        r0, r1 = c * ROWS, (c + 1) * ROWS
        # GCN1 strip: natural node rows, own 1250 columns, 2-way interleave
        a1 = np.zeros((NPAD1, ROWS), np.float16)
        a1[:N] = at16[:, r0:r1]
        a1 = np.ascontiguousarray(
            a1.reshape(NT1, 2, KP, ROWS).transpose(0, 2, 1, 3)
        ).reshape(NT1 * KP, 2, ROWS)

        # GCN2 strip: halo columns, permuted row order
        # [own 1250 (+30 pad) | all cores' rows 0..511 | all cores' 512..1249(+30)]
        acol = np.zeros((N, L), np.float16)
        if c == 0:
            acol[:, HALO:] = at16[:, r0:r1]
        else:
            acol[:, :] = at16[:, r0 - HALO:r1]
        a2 = np.zeros((MT2 * KP, L), np.float16)
        a2[:ROWS] = acol[r0:r1]                       # local block
        for cc in range(CORES):
            if cc == c:
                continue                              # own rows only via local
            g0 = cc * ROWS
            blk1 = NLOC * KP + cc * S1
            blk2 = NLOC * KP + MG1 * KP + cc * S2P
            a2[blk1:blk1 + S1] = acol[g0:g0 + S1]
            a2[blk2:blk2 + S2] = acol[g0 + S1:g0 + ROWS]

        patch = np.zeros((KP, 12), np.float32)
        if c == 0:
            patch[:, 6:10] = -60.0
        else:
            patch[:, 0:6] = 1.0
        in_maps.append({**common, "a1": a1, "a2": a2, "patch": patch})
    return in_maps


def assemble_output(results):
    outs = [r["out_t"].T for r in results]          # each [ROWS, OUT]
    full = np.concatenate(outs, axis=0).astype(np.float32)
    return full[None]                               # [1, N, OUT]


def kernel(**inputs) -> np.ndarray:
    from concourse import bass_utils

    if "nc" not in _CACHE:
        _CACHE["nc"] = build_program()
    nc = _CACHE["nc"]
    in_maps = host_prepare(inputs)
    res = bass_utils.run_bass_kernel_spmd(
        nc, in_maps, core_ids=list(range(CORES)))
    return assemble_output(res.results)


if __name__ == "__main__":
    import reference

    inputs = {k: np.asarray(v) for k, v in reference.setup_inputs().items()}
    out = kernel(**inputs)
    print("kernel out", out.shape, out.dtype)
    np.save("/root/problem/kernel_out.npy", out)


# revision 16
# speedup vs baseline: 1.0419x; 1.0419x over previous
"""DCRNN (2x GCNConv + GRU-over-nodes + Linear) on 8 Trainium2 cores.

Strategy (v2)
-------------
* GCN1 uses associativity: h1 = relu((A^T x) W1 + b1).  The A-contraction
  has only IN_FEAT=64 output rows, so pairs of node K-tiles run as two
  concurrent 64-wide column-group matmuls (tile_position col tiling); the
  64->256 projection contracts the stacked [W1;W1] so the two halves sum
  for free.  GCN1 is DMA-bound (~25 MB of A strip).
* h1 columns are computed in two segments (512 / 738+pad).  Each segment's
  XW2 = h1 @ W2 shard is AllGather'd early (fp16), overlapping the rest of
  GCN1 and the local-tile part of GCN2, so the PE never waits long.
* GCN2 contracts 90 K-tiles in the order: own-local tiles (no gather dep),
  gather-1 tiles, gather-2 tiles.  The host builds the A strip with the
  matching permuted row order (gathered layout), own rows zeroed in the
  gathered blocks.
* GRU over the node sequence: 8 Jacobi fixed-point sweeps; gates via
  matmul + pointwise, the h recurrence applied exactly with the DVE
  affine-scan, split into 3 chained chunks so the next sweep's gate
  matmuls pipeline with the scans.  A 64-row halo decouples the cores.
* Final Linear on the node shard; host concatenates the 8 shards.
"""

import numpy as np

NUM_NODES = 10000
IN_FEAT = 64
HID = 256
OUT = 3
CORES = 8
ROWS = NUM_NODES // CORES          # 1250
HALO = 64
L = ROWS + HALO                    # 1314 local sequence length
SWEEPS = 8
KP = 128

S1 = 512                           # first h1 column segment
S2 = ROWS - S1                     # 738 (padded to 768)
S2P = 768
NT1 = (NUM_NODES + 255) // 256     # 40 paired node tiles for GCN1 (10240)
NG1 = NT1 // 2                     # 20 4-way interleaved GCN1 groups
G1T = S1 // KP                     # 4 tiles/core in gather-1
G2T = S2P // KP                    # 6 tiles/core in gather-2
MG1 = CORES * G1T                  # 32 gather-1 K-tiles
MG2 = CORES * G2T                  # 48 gather-2 K-tiles
MT2 = MG1 + MG2                    # 80 K-tiles for GCN2 (gathered layout)
NLOC = G1T + G2T                   # 10 local XW2 tiles (gather inputs)

_CACHE = {}


def _chunks(total, step=512):
    return [(c, min(c + step, total)) for c in range(0, total, step)]


def build_program():
    import concourse.bass as bass
    import concourse.mybir as mybir
    import concourse.tile as tile
    from concourse import bacc

    f16 = mybir.dt.float16
    f32 = mybir.dt.float32
    AF = mybir.ActivationFunctionType
    ALU = mybir.AluOpType

    nc = bacc.Bacc("TRN2", num_devices=CORES)

    # ---- inputs ----
    # a1 streams for the two GCN1 column segments, 4-way K-tile interleaved
    # so every DMA row is one long contiguous run:
    # [g*128+p, j*cw + c] = A_T[node g*512+j*128+p, segment col c]
    a1s1_d = nc.dram_tensor("a1s1", [NG1 * KP, 4 * S1], f16, kind="ExternalInput")
    a1s2_d = nc.dram_tensor("a1s2", [NG1 * KP, 4 * S2], f16, kind="ExternalInput")
    # a2: GCN2 stream, gathered row order [gather1 | gather2], halo cols.
    a2_d = nc.dram_tensor("a2", [MT2 * KP, L], f16, kind="ExternalInput")
    # x node-major, partition-major packing: [p, (k2*2+j)*64+f]
    xn_d = nc.dram_tensor("xn", [KP, NT1 * 2 * IN_FEAT], f16, kind="ExternalInput")
    w1dup_d = nc.dram_tensor("w1dup", [KP, HID], f16, kind="ExternalInput")
    w2_d = nc.dram_tensor("w2", [HID, HID], f16, kind="ExternalInput")
    wiht_d = nc.dram_tensor("wiht", [HID, 3 * HID], f16, kind="ExternalInput")
    whht_d = nc.dram_tensor("whht", [HID, 3 * HID], f16, kind="ExternalInput")
    fcwt_d = nc.dram_tensor("fcwt", [HID, OUT], f16, kind="ExternalInput")
    ident_d = nc.dram_tensor("ident", [KP, KP], f16, kind="ExternalInput")
    b1c_d = nc.dram_tensor("b1c", [KP, 2], f32, kind="ExternalInput")
    b2c_d = nc.dram_tensor("b2c", [KP, 2], f32, kind="ExternalInput")
    gib_d = nc.dram_tensor("gib", [KP, 6], f32, kind="ExternalInput")
    bhn_d = nc.dram_tensor("bhn", [KP, 2], f32, kind="ExternalInput")
    fcb_d = nc.dram_tensor("fcb", [KP, 1], f32, kind="ExternalInput")
    patch_d = nc.dram_tensor("patch", [KP, 12], f32, kind="ExternalInput")
    out_d = nc.dram_tensor("out_t", [OUT, ROWS], f32, kind="ExternalOutput")

    with tile.TileContext(nc) as tc:
        with (
            tc.tile_pool(name="const", bufs=1) as cpool,
            tc.tile_pool(name="big", bufs=1) as big,
            tc.tile_pool(name="tmp", bufs=4) as tpool,
            tc.tile_pool(name="psxw", bufs=2, space="PSUM") as psxw,
            tc.tile_pool(name="dram", bufs=1, space="DRAM") as dpool,
        ):
            # ---- constants ----
            xn_sb = cpool.tile([KP, NT1, 2, IN_FEAT], f16)
            w1dup_sb = cpool.tile([KP, HID], f16)
            w2_sb = cpool.tile([KP, 2, HID], f16)
            wiht_sb = cpool.tile([KP, 2, 3 * HID], f16)
            whht_sb = cpool.tile([KP, 2, 3 * HID], f16)
            fcwt_sb = cpool.tile([KP, 2, OUT], f16)
            ident_sb = cpool.tile([KP, KP], f16)
            b1c_sb = cpool.tile([KP, 2], f32)
            b2c_sb = cpool.tile([KP, 2], f32)
            gib_sb = cpool.tile([KP, 6], f32)
            bhn_sb = cpool.tile([KP, 2], f32)
            fcb_sb = cpool.tile([KP, 1], f32)
            patch_sb = cpool.tile([KP, 12], f32)

            nc.sync.dma_start(ident_sb[:], ident_d[:])
            nc.sync.dma_start(w1dup_sb[:], w1dup_d[:])
            nc.scalar.dma_start(xn_sb[:], xn_d[:])
            for k in range(2):
                nc.sync.dma_start(w2_sb[:, k, :], w2_d[k * KP:(k + 1) * KP, :])
                nc.sync.dma_start(wiht_sb[:, k, :], wiht_d[k * KP:(k + 1) * KP, :])
                nc.sync.dma_start(whht_sb[:, k, :], whht_d[k * KP:(k + 1) * KP, :])
                nc.sync.dma_start(fcwt_sb[:, k, :], fcwt_d[k * KP:(k + 1) * KP, :])
            nc.sync.dma_start(b1c_sb[:], b1c_d[:])
            nc.sync.dma_start(b2c_sb[:], b2c_d[:])
            nc.sync.dma_start(gib_sb[:], gib_d[:])
            nc.sync.dma_start(bhn_sb[:], bhn_d[:])
            nc.sync.dma_start(fcb_sb[:], fcb_d[:])
            nc.sync.dma_start(patch_sb[:], patch_d[:])

            # tiny AllGather to absorb the first-collective ncfw setup cost
            ccw_in = dpool.tile([CORES, 64], f16)
            ccw_out = dpool.tile([CORES * CORES, 64], f16, addr_space="Shared")
            nc.sync.dma_start(ccw_in[0:8, :], ident_sb[0:8, 0:64])
            nc.gpsimd.collective_compute(
                "AllGather", mybir.AluOpType.bypass,
                replica_groups=[list(range(CORES))],
                ins=[ccw_in.opt()], outs=[ccw_out.opt()])

            # PE warm-up burst so the HAM clock-gate opens before GCN1
            for i in range(16):
                psd = psxw.tile([KP, 512], f32, tag="xwps", name=f"warm_{i}")
                nc.tensor.matmul(psd[:, :KP], ident_sb[:], ident_sb[:],
                                 start=True, stop=True)

            # ================= GCN1: ax = A1^T x, two column segments ====
            a1p_cm = tc.tile_pool(name="a1stream", bufs=5)
            a1pool = a1p_cm.__enter__()
            psax_cm = tc.tile_pool(name="psax", bufs=4, space="PSUM")
            psax = psax_cm.__enter__()
            h1t_sb = big.tile([KP, 2, S1 + S2P], f16)
            # zero the S2 pad columns so XW2 of pad rows is 0 (not garbage)
            nc.vector.memset(h1t_sb[:, 0, S1 + S2:S1 + S2P], 0.0)
            nc.vector.memset(h1t_sb[:, 1, S1 + S2:S1 + S2P], 0.0)

            def gcn1_pass(a1_d, c0, cw, tag):
                chs = _chunks(cw)
                axps = [psax.tile([KP, 512], f32, tag="ax",
                                  name=f"ax_{tag}_{i}")
                        for i in range(len(chs))]
                for g in range(NG1):
                    at = a1pool.tile([KP, 4, cw], f16, tag="a1")
                    eng = nc.sync if g % 2 == 0 else nc.scalar
                    eng.dma_start(at[:], a1_d[g * KP:(g + 1) * KP, :])
                    for jp in range(2):
                        k2 = 2 * g + jp
                        for ci, (cc0, cc1) in enumerate(chs):
                            nc.tensor.matmul(
                                axps[ci][0:64, :cc1 - cc0],
                                xn_sb[:, k2, 0, :],
                                at[:, 2 * jp + 0, cc0:cc1],
                                start=(k2 == 0), stop=(k2 == NT1 - 1),
                                tile_position=(0, 0), skip_group_check=True)
                            nc.tensor.matmul(
                                axps[ci][64:128, :cc1 - cc0],
                                xn_sb[:, k2, 1, :],
                                at[:, 2 * jp + 1, cc0:cc1],
                                start=(k2 == 0), stop=(k2 == NT1 - 1),
                                tile_position=(0, 64), skip_group_check=True)
                # copy ax to SBUF (fp16), project with [W1;W1], relu
                for ci, (cc0, cc1) in enumerate(chs):
                    ccw = cc1 - cc0
                    axs = tpool.tile([KP, 512], f16, tag="axs",
                                     name=f"axs_{tag}_{ci}")
                    nc.scalar.activation(axs[:, :ccw], axps[ci][:, :ccw],
                                         AF.Copy)
                    for mm in range(2):
                        psh = psax.tile([KP, 512], f32, tag="ax",
                                        name=f"psh_{tag}_{ci}_{mm}")
                        nc.tensor.matmul(psh[:, :ccw],
                                         w1dup_sb[:, mm * KP:(mm + 1) * KP],
                                         axs[:, :ccw], start=True, stop=True)
                        nc.scalar.activation(
                            h1t_sb[:, mm, c0 + cc0:c0 + cc1],
                            psh[:, :ccw], AF.Relu,
                            bias=b1c_sb[:, mm:mm + 1])

            gcn1_pass(a1s1_d, 0, S1, "s1")

            # ---- XW2 segment 1 (tiles 0..3), bounce, gather 1 ----
            xw2l_sb = cpool.tile([KP, NLOC, HID], f16)
            bounce1 = dpool.tile([S1, HID], f16)
            bounce2 = dpool.tile([S2P, HID], f16)
            gath1 = dpool.tile([CORES * S1, HID], f16, addr_space="Shared")
            gath2 = dpool.tile([CORES * S2P, HID], f16, addr_space="Shared")

            def xw2_tiles(t0, t1):
                for t in range(t0, t1):
                    ps = psxw.tile([KP, 512], f32, tag="xwps", name=f"xw2_{t}")
                    for k in range(2):
                        nc.tensor.matmul(ps[:, :HID],
                                         h1t_sb[:, k, t * KP:(t + 1) * KP],
                                         w2_sb[:, k, :],
                                         start=(k == 0), stop=(k == 1))
                    if t % 2 == 0:
                        nc.scalar.activation(xw2l_sb[:, t, :], ps[:, :HID],
                                             AF.Copy)
                    else:
                        nc.vector.tensor_copy(xw2l_sb[:, t, :], ps[:, :HID])

            xw2_tiles(0, G1T)
            for t in range(G1T):
                nc.sync.dma_start(bounce1[t * KP:(t + 1) * KP, :],
                                  xw2l_sb[:, t, :])
            nc.gpsimd.collective_compute(
                "AllGather", mybir.AluOpType.bypass,
                replica_groups=[list(range(CORES))],
                ins=[bounce1.opt()], outs=[gath1.opt()])
            # load gathered XW2 back on the gpsimd queue right behind the
            # collective (sync/scalar queues are busy with the A streams)
            xg1_sb = big.tile([KP, MG1, HID], f16)
            xg2_sb = big.tile([KP, MG2, HID], f16)
            for t in range(MG1):
                nc.gpsimd.dma_start(xg1_sb[:, t, :],
                                    gath1[t * KP:(t + 1) * KP, :])

            # ---- GCN1 second column segment, XW2 tiles 4..9, gather 2 ----
            gcn1_pass(a1s2_d, S1, S2, "s2")
            xw2_tiles(G1T, NLOC)
            for t in range(G2T):
                nc.sync.dma_start(bounce2[t * KP:(t + 1) * KP, :],
                                  xw2l_sb[:, G1T + t, :])
            nc.gpsimd.collective_compute(
                "AllGather", mybir.AluOpType.bypass,
                replica_groups=[list(range(CORES))],
                ins=[bounce2.opt()], outs=[gath2.opt()])

            for t in range(MG2):
                nc.gpsimd.dma_start(xg2_sb[:, t, :],
                                    gath2[t * KP:(t + 1) * KP, :])

            a1p_cm.__exit__(None, None, None)
            psax_cm.__exit__(None, None, None)

            # ================= GCN2 over the halo shard ==================
            psG_cm = tc.tile_pool(name="psG", bufs=1, space="PSUM")
            psG = psG_cm.__enter__()
            a2p_cm = tc.tile_pool(name="a2stream", bufs=12)
            a2pool = a2p_cm.__enter__()

            chg2 = _chunks(L)
            ps2 = [[psG.tile([KP, 512], f32, tag=f"G{mm * 3 + ci}",
                             name=f"ps2_{mm}_{ci}")
                    for ci in range(3)] for mm in range(2)]

            def gcn2_k(k, lhs_tile, first, last):
                at = a2pool.tile([KP, L], f16, tag="a2")
                eng = nc.sync if k % 2 == 0 else nc.scalar
                eng.dma_start(at[:], a2_d[k * KP:(k + 1) * KP, :])
                for mm in range(2):
                    lhsT = lhs_tile(mm)
                    for ci, (c0, c1) in enumerate(chg2):
                        nc.tensor.matmul(ps2[mm][ci][:, :c1 - c0], lhsT,
                                         at[:, c0:c1], start=first, stop=last)

            for t in range(MG1):
                gcn2_k(t, lambda mm, t=t: xg1_sb[:, t, mm * KP:(mm + 1) * KP],
                       t == 0, False)
            for t in range(MG2):
                gcn2_k(MG1 + t,
                       lambda mm, t=t: xg2_sb[:, t, mm * KP:(mm + 1) * KP],
                       False, t == MG2 - 1)

            h2t_sb = big.tile([KP, 2, L], f16)
            for mm in range(2):
                for ci, (c0, c1) in enumerate(chg2):
                    nc.scalar.activation(h2t_sb[:, mm, c0:c1],
                                         ps2[mm][ci][:, :c1 - c0], AF.Relu,
                                         bias=b2c_sb[:, mm:mm + 1])

            psG_cm.__exit__(None, None, None)
            a2p_cm.__exit__(None, None, None)
            psg_cm = tc.tile_pool(name="ps", bufs=1, space="PSUM")
            pspool = psg_cm.__enter__()

            # ---- GI = W_ih @ h2T + (b_ih [+ b_hh for r,z]) ----
            ch512 = _chunks(L)
            gi_sb = big.tile([KP, 6, L], f16)
            for c0, c1 in ch512:
                psg = [pspool.tile([KP, 512], f32, tag=f"g{m}",
                                   name=f"psgi_{m}") for m in range(6)]
                for m in range(6):
                    for k in range(2):
                        nc.tensor.matmul(psg[m][:, :c1 - c0],
                                         wiht_sb[:, k, m * KP:(m + 1) * KP],
                                         h2t_sb[:, k, c0:c1],
                                         start=(k == 0), stop=(k == 1))
                    if m % 2 == 0:
                        nc.scalar.activation(gi_sb[:, m, c0:c1],
                                             psg[m][:, :c1 - c0], AF.Identity,
                                             bias=gib_sb[:, m:m + 1])
                    else:
                        nc.vector.tensor_scalar_add(gi_sb[:, m, c0:c1],
                                                    psg[m][:, :c1 - c0],
                                                    gib_sb[:, m:m + 1])
            # per-core GI patch on the first HALO columns
            for m in range(6):
                nc.vector.tensor_scalar(gi_sb[:, m, :HALO], gi_sb[:, m, :HALO],
                                        patch_sb[:, m:m + 1],
                                        patch_sb[:, 6 + m:7 + m],
                                        ALU.mult, ALU.add)

            # ---- GRU fixed-point sweeps, chunk-chained scans ----
            hsh_sb = big.tile([KP, 2, L + 1], f16)
            for mm in range(2):
                nc.vector.memset(hsh_sb[:, mm, :], 0.0)
            for s in range(SWEEPS):
                z_sb = big.tile([KP, 2, L], f16, tag="Z")
                b_sb = big.tile([KP, 2, L], f16, tag="B")
                for ci, (c0, c1) in enumerate(ch512):
                    cw = c1 - c0
                    if ci == 2:
                        # keep-warm: the PE waits here for the previous
                        # sweep's last scan; don't let a HAM window lapse
                        psd = psxw.tile([KP, 512], f32, tag="xwps",
                                        name=f"dwa_{s}")
                        nc.tensor.matmul(psd[:, :512], ident_sb[:],
                                         gi_sb[:, 0, 0:512],
                                         start=True, stop=True)
                    psg = [pspool.tile([KP, 512], f32, tag=f"g{m}",
                                       name=f"psu_{s}_{m}") for m in range(6)]
                    # r,z: identity-load GI then accumulate W_hh @ h_prev
                    for m in range(4):
                        nc.tensor.matmul(psg[m][:, :cw], ident_sb[:],
                                         gi_sb[:, m, c0:c1],
                                         start=True, stop=False)
                    for m in range(6):
                        for k in range(2):
                            nc.tensor.matmul(psg[m][:, :cw],
                                             whht_sb[:, k, m * KP:(m + 1) * KP],
                                             hsh_sb[:, k, c0:c1],
                                             start=(m >= 4 and k == 0),
                                             stop=(k == 1))
                    # gate pointwise ops, op-major so ACT/DVE pipeline
                    r_t = [tpool.tile([KP, 512], f16, tag="r",
                                      name=f"r_{s}_{ci}_{m}") for m in range(2)]
                    t_t = [tpool.tile([KP, 512], f16, tag="t",
                                      name=f"t_{s}_{ci}_{m}") for m in range(2)]
                    un_t = [tpool.tile([KP, 512], f16, tag="un",
                                       name=f"un_{s}_{ci}_{m}") for m in range(2)]
                    n_t = [tpool.tile([KP, 512], f16, tag="n",
                                      name=f"n_{s}_{ci}_{m}") for m in range(2)]
                    for mm in range(2):
                        nc.scalar.activation(r_t[mm][:, :cw], psg[mm][:, :cw],
                                             AF.Sigmoid)
                    for mm in range(2):
                        nc.scalar.activation(z_sb[:, mm, c0:c1],
                                             psg[2 + mm][:, :cw], AF.Sigmoid)
                    for mm in range(2):
                        nc.vector.scalar_tensor_tensor(
                            t_t[mm][:, :cw], psg[4 + mm][:, :cw],
                            bhn_sb[:, mm:mm + 1], r_t[mm][:, :cw],
                            ALU.add, ALU.mult)
                    for mm in range(2):
                        nc.vector.tensor_add(un_t[mm][:, :cw], t_t[mm][:, :cw],
                                             gi_sb[:, 4 + mm, c0:c1])
                    for mm in range(2):
                        nc.scalar.activation(n_t[mm][:, :cw], un_t[mm][:, :cw],
                                             AF.Tanh)
                    for mm in range(2):
                        nc.vector.scalar_tensor_tensor(
                            b_sb[:, mm, c0:c1], z_sb[:, mm, c0:c1], 1.0,
                            n_t[mm][:, :cw], ALU.subtract, ALU.mult)
                    # chained chunk scans (exact affine recurrence)
                    for mm in range(2):
                        nc.vector.tensor_tensor_scan(
                            hsh_sb[:, mm, 1 + c0:1 + c1],
                            z_sb[:, mm, c0:c1], b_sb[:, mm, c0:c1],
                            0.0 if ci == 0 else hsh_sb[:, mm, c0:c0 + 1],
                            ALU.mult, ALU.subtract)

            psg_cm.__exit__(None, None, None)

            # ---- final Linear on the real rows (skip halo) ----
            out_sb = cpool.tile([4, ROWS], f32)
            for c0, c1 in _chunks(ROWS):
                cw = c1 - c0
                psf = psxw.tile([KP, 512], f32, tag="xwps")
                for k in range(2):
                    nc.tensor.matmul(psf[:OUT, :cw], fcwt_sb[:, k, :],
                                     hsh_sb[:, k, HALO + 1 + c0:HALO + 1 + c1],
                                     start=(k == 0), stop=(k == 1))
                nc.scalar.activation(out_sb[:OUT, c0:c1], psf[:OUT, :cw],
                                     AF.Identity, bias=fcb_sb[:OUT, :])
            nc.sync.dma_start(out_d[:], out_sb[:OUT, :])

    nc.compile()
    return nc


def host_prepare(inputs):
    """Build the per-core input maps from the full problem inputs."""
    x = np.asarray(inputs["x"], np.float32)
    ei = np.asarray(inputs["edge_index"])
    W1 = np.asarray(inputs["W1"], np.float32)
    b1 = np.asarray(inputs["b1"], np.float32)
    W2 = np.asarray(inputs["W2"], np.float32)
    b2 = np.asarray(inputs["b2"], np.float32)
    W_ih = np.asarray(inputs["W_ih"], np.float32)
    W_hh = np.asarray(inputs["W_hh"], np.float32)
    b_ih = np.asarray(inputs["b_ih"], np.float32)
    b_hh = np.asarray(inputs["b_hh"], np.float32)
    fc_w = np.asarray(inputs["fc_w"], np.float32)
    fc_b = np.asarray(inputs["fc_b"], np.float32)

    N = NUM_NODES
    src, dst = ei[0].astype(np.int64), ei[1].astype(np.int64)
    deg = np.bincount(dst, minlength=N).astype(np.float64) + 1.0
    dinv = 1.0 / np.sqrt(deg)
    # A_T[s, d] = normalization weight of edge s->d (plus self loops)
    at = np.zeros((N, N), np.float32)
    np.add.at(at, (src, dst), (dinv[src] * dinv[dst]).astype(np.float32))
    idx = np.arange(N)
    at[idx, idx] += (dinv * dinv).astype(np.float32)
    at16 = at.astype(np.float16)
    del at

    # x node-major, partition-major packing: [p, (k2*2+j)*64+f]
    NPAD1 = NT1 * 256                    # 10240
    xn = np.zeros((NPAD1, IN_FEAT), np.float16)
    xn[:N] = x.astype(np.float16)
    xn = np.ascontiguousarray(
        xn.reshape(NT1 * 2, KP, IN_FEAT).transpose(1, 0, 2)
    ).reshape(KP, NT1 * 2 * IN_FEAT)

    common = {
        "xn": xn,
        "w1dup": np.concatenate([W1, W1], axis=0).astype(np.float16),
        "w2": W2.astype(np.float16),
        "wiht": W_ih.T.astype(np.float16),
        "whht": W_hh.T.astype(np.float16),
        "fcwt": fc_w.T.astype(np.float16),
        "ident": np.eye(KP, dtype=np.float16),
        "b1c": b1.reshape(2, KP).T.astype(np.float32).copy(),
        "b2c": b2.reshape(2, KP).T.astype(np.float32).copy(),
        "gib": (b_ih + np.concatenate([b_hh[:2 * HID],
                                       np.zeros(HID, np.float32)])
                ).reshape(6, KP).T.astype(np.float32).copy(),
        "bhn": b_hh[2 * HID:].reshape(2, KP).T.astype(np.float32).copy(),
        "fcb": np.concatenate([fc_b, np.zeros(KP - OUT, np.float32)]
                              ).reshape(KP, 1),
    }

    in_maps = []
    for c in range(CORES):
        r0, r1 = c * ROWS, (c + 1) * ROWS
        # GCN1 strips: natural node rows, own columns, 4-way interleave per
        # column segment so each DMA row is one contiguous run
        a1 = np.zeros((NPAD1, ROWS), np.float16)
        a1[:N] = at16[:, r0:r1]

        def ileave(seg):                              # [10240, cw] -> 4-way
            cw = seg.shape[1]
            return np.ascontiguousarray(
                seg.reshape(NG1, 4, KP, cw).transpose(0, 2, 1, 3)
            ).reshape(NG1 * KP, 4 * cw)

        a1s1 = ileave(a1[:, :S1])
        a1s2 = ileave(a1[:, S1:])

        # GCN2 strip: halo columns, gathered row order
        # [all cores' rows 0..511 | all cores' rows 512..1249 (+30 pad)]
        acol = np.zeros((N, L), np.float16)
        if c == 0:
            acol[:, HALO:] = at16[:, r0:r1]
        else:
            acol[:, :] = at16[:, r0 - HALO:r1]
        a2 = np.zeros((MT2 * KP, L), np.float16)
        for cc in range(CORES):
            g0 = cc * ROWS
            blk1 = cc * S1
            blk2 = MG1 * KP + cc * S2P
            a2[blk1:blk1 + S1] = acol[g0:g0 + S1]
            a2[blk2:blk2 + S2] = acol[g0 + S1:g0 + ROWS]

        patch = np.zeros((KP, 12), np.float32)
        if c == 0:
            patch[:, 6:10] = -60.0
        else:
            patch[:, 0:6] = 1.0
        in_maps.append({**common, "a1s1": a1s1, "a1s2": a1s2, "a2": a2,
                        "patch": patch})
    return in_maps


def assemble_output(results):
    outs = [r["out_t"].T for r in results]          # each [ROWS, OUT]
    full = np.concatenate(outs, axis=0).astype(np.float32)
    return full[None]                               # [1, N, OUT]


def kernel(**inputs) -> np.ndarray:
    from concourse import bass_utils

    if "nc" not in _CACHE:
        _CACHE["nc"] = build_program()
    nc = _CACHE["nc"]
    in_maps = host_prepare(inputs)
    res = bass_utils.run_bass_kernel_spmd(
        nc, in_maps, core_ids=list(range(CORES)))
    return assemble_output(res.results)


if __name__ == "__main__":
    import reference

    inputs = {k: np.asarray(v) for k, v in reference.setup_inputs().items()}
    out = kernel(**inputs)
    print("kernel out", out.shape, out.dtype)
    np.save("/root/problem/kernel_out.npy", out)
